# revision 7
# baseline (speedup 1.0000x reference)
"""Conv2D 3x3 (stride 1, pad 1) NCHW/OIHW, data-parallel over 8 NeuronCores.

Full inputs: x (16,32,224,224) f32, weight (64,32,3,3) f32, bias (64,) f32.
Full output: (16,64,224,224) f32.

Raw-Bass SPMD kernel, per core (2 images), per 28-row block:
  - The im2col buffer XB[img][p = dy*32 + ic, s, c] = xpad[img, ic, i0+s+dy, c]
    for dy=0,1 is DMA'd DIRECTLY from HBM.  Each load's DRAM-side AP has the
    32-wide ic dim outermost, so the descriptor generator sprays it
    round-robin across all 16 SDMA engines (the outermost source dim is what
    gets split).  Loads are split into <=15-row pieces so every DMA packet
    stays <=6.8KB: the SDMA engines move ~25GB/s on ~6KB packets but drop to
    ~14GB/s on >8KB packets.  The dy=2 replica is one wide DVE copy from the
    dy=1 group (which loads 29 row-slots so slot s+1 covers row i0+2+s),
    keeping HBM input traffic at 2x the unique bytes instead of 3x.
  - Engine assignment decouples the three pipelines so PE never waits on a
    chain that leads back through input-DMA arrival:
      GpSimd: issues all input DMAs (its own hardware queue, separate from
              the store queue, so loads never sit behind stores in a FIFO).
      DVE:    dy=2 copies for block b, then PSUM evac of row-pairs 7..13 of
              block b-1 (tensor_scalar_add w/ per-partition bias, f32->bf16).
      Scalar: PSUM evac of row-pairs 0..6 (activation w/ bias).
      SP:     output stores; each store half waits on exactly one evac sem.
  - Each output row-pair = 3 PSUM-accumulated matmuls (K=96, M=64, N=448),
    dx realized as a free-dim offset.  The two images ride different PE
    column groups (PSUM partitions 0-63 / 64-127) and overlap in the array.
    This is SBUF->PE rhs-port bound (~28.9M rhs element reads / 128 lanes
    @2.8GHz ~= 80us per core), the structural floor for M=64.
  - Output rides HBM as bf16 (halves store traffic; measured end-to-end rel
    err ~3.6e-3 vs the 2e-2 gate) and is widened to f32 on the host.
  - All cross-engine sync is explicit semaphores; DMA-sem waiters always
    wait the full +16-per-DMA count (sound under out-of-order per-SDMA
    completion).  The local walrus rejects multi-wait instructions, so every
    wait is its own instruction.
"""

import sys

sys.path.insert(0, "/opt/trn_rl_repo")

from contextlib import ExitStack

import numpy as np

import concourse.bass as bass
from concourse import mybir
from concourse.bass_utils import run_bass_kernel_spmd

N_CORES = 8
IMGS_PER_CORE = 2
IC, OC, H, W = 32, 64, 224, 224
HP, WP = 226, 226  # padded
BLK = 28  # output rows per block
N_BLK = H // BLK
PPB = BLK // 2  # row-pairs per block (14)
SLOTS = BLK + 1  # xb row-slots (dy=1 group loads one extra row for the dy=2 copy)
XR = 4  # xb ring depth
OR = 2  # out ring depth
NPS = 8  # psum banks in rotation
EVS_PPB = 8  # row-pairs 0..7 per block evacuated by ScalarE
EVV_PPB = 6  # row-pairs 8..13 per block evacuated by DVE

DT_MODE = "bf16"  # kept for test.py compat; only bf16 is supported

TRACE = False  # test.py can flip this to get LAST_EXEC_NS
LAST_EXEC_NS = None
LAST_RESULTS = None

_nc_cache = {}


def _install_ntff_shim():
    """The agent image's antenv lacks axon_hooks; recreate the NTFF profile
    hook via ctypes against libaxon_pjrt.so (same ABI trn_boot.py uses)."""
    try:
        import antenv.axon_hooks  # noqa: F401

        return
    except ImportError:
        pass
    import contextlib
    import ctypes
    import types

    so_path = "/opt/axon/libaxon_pjrt.so"
    lib = ctypes.CDLL(so_path)
    if not hasattr(lib, "axon_start_nrt_profile"):
        return
    lib.axon_start_nrt_profile.argtypes = [
        ctypes.POINTER(ctypes.c_int64),
        ctypes.c_size_t,
    ]
    lib.axon_start_nrt_profile.restype = ctypes.c_int64
    lib.axon_stop_nrt_profile.argtypes = [ctypes.c_char_p]
    lib.axon_stop_nrt_profile.restype = ctypes.c_int64

    @contextlib.contextmanager
    def _hook(output_dir, device_ids):
        import jax

        jax.devices()
        if device_ids:
            ids = (ctypes.c_int64 * len(device_ids))(*device_ids)
            rc = lib.axon_start_nrt_profile(ids, len(device_ids))
        else:
            rc = lib.axon_start_nrt_profile(None, 0)
        if rc != 0:
            raise RuntimeError(f"axon_start_nrt_profile rc={rc}")
        try:
            yield
        finally:
            n = lib.axon_stop_nrt_profile(str(output_dir).encode())
            print(f"ntff profile: {n} file(s) written to {output_dir}")

    mod = types.ModuleType("antenv.axon_hooks")
    mod.get_axon_ntff_profile_hook = lambda: _hook
    mod.set_axon_ntff_profile_hook = lambda h: None
    import antenv

    sys.modules["antenv.axon_hooks"] = mod
    antenv.axon_hooks = mod


def _build_nc() -> bass.Bass:
    f32 = mybir.dt.float32
    bf16 = mybir.dt.bfloat16

    nc = bass.Bass()
    x = nc.dram_tensor("x", [IMGS_PER_CORE, IC, HP, WP], bf16, kind="ExternalInput")
    wt = nc.dram_tensor("wt", [96, 3, OC], bf16, kind="ExternalInput")
    bias = nc.dram_tensor("bias", [128, 1], f32, kind="ExternalInput")
    y = nc.dram_tensor("y", [IMGS_PER_CORE, OC, H, W], bf16, kind="ExternalOutput")

    ctx = ExitStack()
    wt_sb = ctx.enter_context(nc.sbuf_tensor("wt_sb", [96, 3, OC], bf16))
    bias_sb = ctx.enter_context(nc.sbuf_tensor("bias_sb", [128, 1], f32))
    xb = [
        [
            ctx.enter_context(nc.sbuf_tensor(f"xb_{i}_{r}", [96, SLOTS, WP], bf16))
            for r in range(XR)
        ]
        for i in range(IMGS_PER_CORE)
    ]
    outb = [
        ctx.enter_context(nc.sbuf_tensor(f"outb_{s}", [128, BLK, W], bf16))
        for s in range(OR)
    ]
    ps = [
        ctx.enter_context(nc.psum_tensor(f"ps_{k}", [128, 2, W], f32))
        for k in range(NPS)
    ]

    s_wt = ctx.enter_context(nc.semaphore("s_wt"))
    s_bias = ctx.enter_context(nc.semaphore("s_bias"))
    s_xb = [
        [ctx.enter_context(nc.semaphore(f"s_xb_{i}_{r}")) for r in range(XR)]
        for i in range(IMGS_PER_CORE)
    ]
    s_yo = [
        [ctx.enter_context(nc.semaphore(f"s_yo_{s}_{h}")) for h in range(2)]
        for s in range(OR)
    ]
    s_cp = ctx.enter_context(nc.semaphore("s_cp"))
    s_mm = ctx.enter_context(nc.semaphore("s_mm"))
    s_evs = ctx.enter_context(nc.semaphore("s_evs"))
    s_evv = ctx.enter_context(nc.semaphore("s_evv"))

    st_img = IC * HP * WP
    st_ic = HP * WP

    # (dy, slot0, n_rows) pieces: <=15 rows keeps every packet <=6.8KB.
    DMA_PIECES = [(0, 0, 14), (0, 14, 14), (1, 0, 15), (1, 15, 14)]

    def piece_src(img, i0, dy, s0, n_rows):
        # partition = ic (32-wide, outermost -> 16-engine DMA spray);
        # free (s, c); each partition reads n_rows*WP contiguous elements.
        return bass.AP(
            tensor=x[0, 0, 0:1, 0:1].tensor,
            offset=img * st_img + (i0 + dy + s0) * WP,
            ap=[[st_ic, IC], [WP, n_rows], [1, WP]],
        )

    with ctx, nc.Block() as block:

        def _issue_inputs(eng, b, img):
            r = b % XR
            i0 = b * BLK
            if b >= XR:
                # xb slot reuse: PE matmuls of block b-XR done (the dy=2
                # copies of b-XR precede PE b-XR via s_cp, so this also
                # covers the copy's read of the dy=1 rows).
                eng.wait_ge(s_mm, PPB * (b - XR + 1))
            for dy, s0, n_rows in DMA_PIECES:
                eng.dma_start(
                    out=xb[img][r][dy * 32 : (dy + 1) * 32, s0 : s0 + n_rows, :],
                    in_=piece_src(img, i0, dy, s0, n_rows),
                ).then_inc(s_xb[img][r], 16)

        @block.gpsimd
        def _(g):
            # img0 input DMA issue: its own hardware queue, so input
            # transfers never sit behind output stores on a shared FIFO.
            for b in range(N_BLK):
                _issue_inputs(g, b, 0)

        @block.sync
        def _(sync):
            def emit_out(b):
                i0 = b * BLK
                ob = outb[b % OR]
                # h=0: rows 0..13 = pairs 0..6, all ScalarE evacs
                sync.wait_ge(s_evs, EVS_PPB * b + 7)
                sync.dma_start(
                    out=y[:, :, i0 : i0 + PPB, :],
                    in_=ob[:, 0:PPB, :],
                ).then_inc(s_yo[b % OR][0], 16)
                # h=1: rows 14..27 = pair 7 (ScalarE) + pairs 8..13 (DVE)
                sync.wait_ge(s_evs, EVS_PPB * (b + 1))
                sync.wait_ge(s_evv, EVV_PPB * (b + 1))
                sync.dma_start(
                    out=y[:, :, i0 + PPB : i0 + BLK, :],
                    in_=ob[:, PPB:BLK, :],
                ).then_inc(s_yo[b % OR][1], 16)

            sync.dma_start(out=wt_sb[:, :, :], in_=wt[:, :, :]).then_inc(s_wt, 16)
            sync.dma_start(out=bias_sb[:, :], in_=bias[:, :]).then_inc(s_bias, 16)
            for b in range(N_BLK):
                _issue_inputs(sync, b, 1)
                if b >= 1:
                    emit_out(b - 1)
            emit_out(N_BLK - 1)
            for s in range(OR):
                n_uses = len([bb for bb in range(N_BLK) if bb % OR == s])
                for h in range(2):
                    sync.wait_ge(s_yo[s][h], 16 * n_uses)

        def _evac_dve(v, b):
            # row-pairs 8..13 of block b
            ob = outb[b % OR]
            for p in range(PPB - EVV_PPB, PPB):
                gp = b * PPB + p
                if b >= OR and p == PPB - EVV_PPB:
                    v.wait_ge(s_yo[b % OR][1], 16 * ((b - OR) // OR + 1))
                v.wait_ge(s_mm, gp + 1)
                v.tensor_scalar_add(
                    ob[:, 2 * p : 2 * p + 2, :],
                    ps[gp % NPS][:, :, :],
                    bias_sb[:, :],
                ).then_inc(s_evv, 1)

        @block.vector
        def _(v):
            v.wait_ge(s_bias, 16)
            # copies for block b run first (inputs arrive well ahead), then
            # evacs of block b-1 drain as PE produces them -- so a late
            # input can only delay PE block b's start, never PSUM recycling.
            for b in range(N_BLK):
                r = b % XR
                for img in range(IMGS_PER_CORE):
                    v.wait_ge(s_xb[img][r], 16 * len(DMA_PIECES) * (b // XR + 1))
                for img in range(IMGS_PER_CORE):
                    # dy=2 im2col replica: xb[64:96, s] = xb[32:64, s+1]
                    v.tensor_copy(
                        out=xb[img][r][64:96, 0:BLK, :],
                        in_=xb[img][r][32:64, 1 : BLK + 1, :],
                    ).then_inc(s_cp, 1)
                if b >= 1:
                    _evac_dve(v, b - 1)
            _evac_dve(v, N_BLK - 1)

        @block.tensor
        def _(t):
            t.wait_ge(s_wt, 16)
            for b in range(N_BLK):
                r = b % XR
                for img in range(IMGS_PER_CORE):
                    t.wait_ge(s_xb[img][r], 16 * len(DMA_PIECES) * (b // XR + 1))
                t.wait_ge(s_cp, IMGS_PER_CORE * (b + 1))
                for p in range(PPB):
                    gp = b * PPB + p
                    if gp >= NPS:
                        # psum bank reuse: evac of row-pair gp-NPS done
                        tb, tp = divmod(gp - NPS, PPB)
                        if tp < EVS_PPB:
                            t.wait_ge(s_evs, EVS_PPB * tb + tp + 1)
                        else:
                            t.wait_ge(s_evv, EVV_PPB * tb + tp - EVS_PPB + 1)
                    bank = ps[gp % NPS]
                    b0 = 2 * p
                    last = None
                    for dx in range(3):
                        for img in range(IMGS_PER_CORE):
                            last = nc.tensor.matmul(
                                bank[img * OC : (img + 1) * OC, :, :],
                                wt_sb[:, dx, :],
                                xb[img][r][:, b0 : b0 + 2, dx : dx + W],
                                start=dx == 0,
                                stop=dx == 2,
                                skip_group_check=True,
                            )
                    last.then_inc(s_mm, 1)

        @block.scalar
        def _(sc):
            sc.wait_ge(s_bias, 16)
            for b in range(N_BLK):
                ob = outb[b % OR]
                for p in range(0, EVS_PPB):
                    gp = b * PPB + p
                    if b >= OR and p == 0:
                        sc.wait_ge(s_yo[b % OR][0], 16 * ((b - OR) // OR + 1))
                    if b >= OR and p == 7:
                        sc.wait_ge(s_yo[b % OR][1], 16 * ((b - OR) // OR + 1))
                    sc.wait_ge(s_mm, gp + 1)
                    sc.activation(
                        ob[:, 2 * p : 2 * p + 2, :],
                        ps[gp % NPS][:, :, :],
                        mybir.ActivationFunctionType.Identity,
                        bias=bias_sb[:, :],
                    ).then_inc(s_evs, 1)

    return nc


def _get_nc() -> bass.Bass:
    if "nc" not in _nc_cache:
        _nc_cache["nc"] = _build_nc()
    return _nc_cache["nc"]


def kernel(x: np.ndarray, weight: np.ndarray, bias: np.ndarray) -> np.ndarray:
    global LAST_EXEC_NS, LAST_RESULTS
    import ml_dtypes

    n = x.shape[0]
    assert n == N_CORES * IMGS_PER_CORE

    in_np = ml_dtypes.bfloat16
    xp = np.zeros((n, IC, HP, WP), dtype=in_np)
    xp[:, :, 1 : H + 1, 1 : W + 1] = x
    # WT[dy*32+ic, dx, oc] = weight[oc, ic, dy, dx]
    wt = np.ascontiguousarray(weight.transpose(2, 1, 3, 0).reshape(96, 3, OC)).astype(
        in_np
    )
    b2 = np.ascontiguousarray(np.tile(bias.reshape(OC, 1), (2, 1))).astype(np.float32)

    nc = _get_nc()
    in_maps = [
        {
            "x": np.ascontiguousarray(xp[i * IMGS_PER_CORE : (i + 1) * IMGS_PER_CORE]),
            "wt": wt,
            "bias": b2,
        }
        for i in range(N_CORES)
    ]
    if TRACE:
        _install_ntff_shim()
    res = run_bass_kernel_spmd(nc, in_maps, core_ids=list(range(N_CORES)), trace=TRACE)
    LAST_EXEC_NS = res.exec_time_ns
    LAST_RESULTS = res
    y = np.concatenate([r["y"] for r in res.results], axis=0)
    return y.astype(np.float32)


# revision 8
# speedup vs baseline: 1.4594x; 1.4594x over previous
"""Conv2D 3x3 (stride 1, pad 1) NCHW/OIHW, data-parallel over 8 NeuronCores.

Full inputs: x (16,32,224,224) f32, weight (64,32,3,3) f32, bias (64,) f32.
Full output: (16,64,224,224) f32.

Raw-Bass SPMD kernel, per core (2 images), per 28-row block:
  - The im2col buffer XB[img][p = dy*32 + ic, s, c] = xpad[img, ic, i0+s+dy, c]
    for dy=0,1 is DMA'd DIRECTLY from HBM.  Each load's DRAM-side AP has the
    32-wide ic dim outermost, so the descriptor generator sprays it
    round-robin across all 16 SDMA engines (the outermost source dim is what
    gets split).  Loads are split into <=15-row pieces so every DMA packet
    stays <=6.8KB: the SDMA engines move ~25GB/s on ~6KB packets but drop to
    ~14GB/s on >8KB packets.  The dy=2 replica is one wide DVE copy from the
    dy=1 group (which loads 29 row-slots so slot s+1 covers row i0+2+s),
    keeping HBM input traffic at 2x the unique bytes instead of 3x.
  - Engine assignment decouples the three pipelines so PE never waits on a
    chain that leads back through input-DMA arrival:
      GpSimd: issues all input DMAs (its own hardware queue, separate from
              the store queue, so loads never sit behind stores in a FIFO).
      DVE:    dy=2 copies for block b, then PSUM evac of row-pairs 7..13 of
              block b-1 (tensor_scalar_add w/ per-partition bias, f32->bf16).
      Scalar: PSUM evac of row-pairs 0..6 (activation w/ bias).
      SP:     output stores; each store half waits on exactly one evac sem.
  - Each output row-pair = 3 PSUM-accumulated matmuls (K=96, M=64, N=448),
    dx realized as a free-dim offset.  The two images ride different PE
    column groups (PSUM partitions 0-63 / 64-127) and overlap in the array.
    This is SBUF->PE rhs-port bound (~28.9M rhs element reads / 128 lanes
    @2.8GHz ~= 80us per core), the structural floor for M=64.
  - Output rides HBM as bf16 (halves store traffic; measured end-to-end rel
    err ~3.6e-3 vs the 2e-2 gate) and is widened to f32 on the host.
  - All cross-engine sync is explicit semaphores; DMA-sem waiters always
    wait the full +16-per-DMA count (sound under out-of-order per-SDMA
    completion).  The local walrus rejects multi-wait instructions, so every
    wait is its own instruction.
"""

import sys

sys.path.insert(0, "/opt/trn_rl_repo")

from contextlib import ExitStack

import numpy as np

import concourse.bass as bass
from concourse import mybir
from concourse.bass_utils import run_bass_kernel_spmd

N_CORES = 8
IMGS_PER_CORE = 2
IC, OC, H, W = 32, 64, 224, 224
HP, WP = 226, 226  # padded
BLK = 28  # output rows per block
N_BLK = H // BLK
PPB = BLK // 2  # row-pairs per block (14)
SLOTS = BLK + 1  # xb row-slots (dy=1 group loads one extra row for the dy=2 copy)
XR = 4  # xb ring depth
OR = 3  # out ring depth
NPS = 8  # psum banks in rotation
EVS_PPB = 8  # row-pairs 0..7 per block evacuated by ScalarE
EVV_PPB = 6  # row-pairs 8..13 per block evacuated by DVE

DT_MODE = "bf16"  # kept for test.py compat; only bf16 is supported

TRACE = False  # test.py can flip this to get LAST_EXEC_NS
LAST_EXEC_NS = None
LAST_RESULTS = None

_nc_cache = {}


def _install_ntff_shim():
    """The agent image's antenv lacks axon_hooks; recreate the NTFF profile
    hook via ctypes against libaxon_pjrt.so (same ABI trn_boot.py uses)."""
    try:
        import antenv.axon_hooks  # noqa: F401

        return
    except ImportError:
        pass
    import contextlib
    import ctypes
    import types

    so_path = "/opt/axon/libaxon_pjrt.so"
    lib = ctypes.CDLL(so_path)
    if not hasattr(lib, "axon_start_nrt_profile"):
        return
    lib.axon_start_nrt_profile.argtypes = [
        ctypes.POINTER(ctypes.c_int64),
        ctypes.c_size_t,
    ]
    lib.axon_start_nrt_profile.restype = ctypes.c_int64
    lib.axon_stop_nrt_profile.argtypes = [ctypes.c_char_p]
    lib.axon_stop_nrt_profile.restype = ctypes.c_int64

    @contextlib.contextmanager
    def _hook(output_dir, device_ids):
        import jax

        jax.devices()
        if device_ids:
            ids = (ctypes.c_int64 * len(device_ids))(*device_ids)
            rc = lib.axon_start_nrt_profile(ids, len(device_ids))
        else:
            rc = lib.axon_start_nrt_profile(None, 0)
        if rc != 0:
            raise RuntimeError(f"axon_start_nrt_profile rc={rc}")
        try:
            yield
        finally:
            n = lib.axon_stop_nrt_profile(str(output_dir).encode())
            print(f"ntff profile: {n} file(s) written to {output_dir}")

    mod = types.ModuleType("antenv.axon_hooks")
    mod.get_axon_ntff_profile_hook = lambda: _hook
    mod.set_axon_ntff_profile_hook = lambda h: None
    import antenv

    sys.modules["antenv.axon_hooks"] = mod
    antenv.axon_hooks = mod


def _build_nc() -> bass.Bass:
    f32 = mybir.dt.float32
    bf16 = mybir.dt.bfloat16

    nc = bass.Bass()
    x = nc.dram_tensor("x", [IMGS_PER_CORE, IC, HP, WP], bf16, kind="ExternalInput")
    wt = nc.dram_tensor("wt", [96, 3, OC], bf16, kind="ExternalInput")
    bias = nc.dram_tensor("bias", [128, 1], f32, kind="ExternalInput")
    y = nc.dram_tensor("y", [IMGS_PER_CORE, OC, H, W], bf16, kind="ExternalOutput")

    ctx = ExitStack()
    wt_sb = ctx.enter_context(nc.sbuf_tensor("wt_sb", [96, 3, OC], bf16))
    bias_sb = ctx.enter_context(nc.sbuf_tensor("bias_sb", [128, 1], f32))
    xb = [
        [
            ctx.enter_context(nc.sbuf_tensor(f"xb_{i}_{r}", [96, SLOTS, WP], bf16))
            for r in range(XR)
        ]
        for i in range(IMGS_PER_CORE)
    ]
    outb = [
        ctx.enter_context(nc.sbuf_tensor(f"outb_{s}", [128, BLK, W], bf16))
        for s in range(OR)
    ]
    ps = [
        ctx.enter_context(nc.psum_tensor(f"ps_{k}", [128, 2, W], f32))
        for k in range(NPS)
    ]

    s_wt = ctx.enter_context(nc.semaphore("s_wt"))
    s_bias = ctx.enter_context(nc.semaphore("s_bias"))
    s_xb = [
        [ctx.enter_context(nc.semaphore(f"s_xb_{i}_{r}")) for r in range(XR)]
        for i in range(IMGS_PER_CORE)
    ]
    s_yo = [
        [ctx.enter_context(nc.semaphore(f"s_yo_{s}_{h}")) for h in range(2)]
        for s in range(OR)
    ]
    s_cp = ctx.enter_context(nc.semaphore("s_cp"))
    s_mm = ctx.enter_context(nc.semaphore("s_mm"))
    s_evs = ctx.enter_context(nc.semaphore("s_evs"))
    s_evv = ctx.enter_context(nc.semaphore("s_evv"))

    st_img = IC * HP * WP
    st_ic = HP * WP

    # (dy, slot0, n_rows) pieces: <=15 rows keeps every packet <=6.8KB.
    DMA_PIECES = [(0, 0, 14), (0, 14, 14), (1, 0, 15), (1, 15, 14)]

    def piece_src(img, i0, dy, s0, n_rows):
        # partition = ic (32-wide, outermost -> 16-engine DMA spray);
        # free (s, c); each partition reads n_rows*WP contiguous elements.
        return bass.AP(
            tensor=x[0, 0, 0:1, 0:1].tensor,
            offset=img * st_img + (i0 + dy + s0) * WP,
            ap=[[st_ic, IC], [WP, n_rows], [1, WP]],
        )

    with ctx, nc.Block() as block:

        def _issue_inputs(eng, b, img):
            r = b % XR
            i0 = b * BLK
            if b >= XR and img == 0:
                # xb slot reuse: PE matmuls of block b-XR done (the dy=2
                # copies of b-XR precede PE b-XR via s_cp, so this also
                # covers the copy's read of the dy=1 rows).
                eng.wait_ge(s_mm, PPB * (b - XR + 1))
            for dy, s0, n_rows in DMA_PIECES:
                eng.dma_start(
                    out=xb[img][r][dy * 32 : (dy + 1) * 32, s0 : s0 + n_rows, :],
                    in_=piece_src(img, i0, dy, s0, n_rows),
                ).then_inc(s_xb[img][r], 16)

        @block.gpsimd
        def _(g):
            # all input DMA issue: its own hardware queue, so input
            # transfers never sit behind output stores on a shared FIFO,
            # and issue never sits behind store/evac waits (SP couples
            # store emission to evac progress; putting input issue there
            # feeds input latency back into the PE pipeline).
            for b in range(N_BLK):
                for img in range(IMGS_PER_CORE):
                    _issue_inputs(g, b, img)

        @block.sync
        def _(sync):
            def emit_out(b):
                i0 = b * BLK
                ob = outb[b % OR]
                # h=0: rows 0..13 = pairs 0..6, all ScalarE evacs
                sync.wait_ge(s_evs, EVS_PPB * b + 7)
                sync.dma_start(
                    out=y[:, :, i0 : i0 + PPB, :],
                    in_=ob[:, 0:PPB, :],
                ).then_inc(s_yo[b % OR][0], 16)
                # h=1: rows 14..27 = pair 7 (ScalarE) + pairs 8..13 (DVE)
                sync.wait_ge(s_evs, EVS_PPB * (b + 1))
                sync.wait_ge(s_evv, EVV_PPB * (b + 1))
                sync.dma_start(
                    out=y[:, :, i0 + PPB : i0 + BLK, :],
                    in_=ob[:, PPB:BLK, :],
                ).then_inc(s_yo[b % OR][1], 16)

            sync.dma_start(out=wt_sb[:, :, :], in_=wt[:, :, :]).then_inc(s_wt, 16)
            sync.dma_start(out=bias_sb[:, :], in_=bias[:, :]).then_inc(s_bias, 16)
            for b in range(N_BLK):
                if b >= 1:
                    emit_out(b - 1)
            emit_out(N_BLK - 1)
            for s in range(OR):
                n_uses = len([bb for bb in range(N_BLK) if bb % OR == s])
                for h in range(2):
                    sync.wait_ge(s_yo[s][h], 16 * n_uses)

        def _evac_dve(v, b):
            # row-pairs 8..13 of block b
            ob = outb[b % OR]
            for p in range(PPB - EVV_PPB, PPB):
                gp = b * PPB + p
                if b >= OR and p == PPB - EVV_PPB:
                    v.wait_ge(s_yo[b % OR][1], 16 * ((b - OR) // OR + 1))
                v.wait_ge(s_mm, gp + 1)
                v.tensor_scalar_add(
                    ob[:, 2 * p : 2 * p + 2, :],
                    ps[gp % NPS][:, :, :],
                    bias_sb[:, :],
                ).then_inc(s_evv, 1)

        @block.vector
        def _(v):
            v.wait_ge(s_bias, 16)
            # copies for block b run first (inputs arrive well ahead), then
            # evacs of block b-1 drain as PE produces them -- so a late
            # input can only delay PE block b's start, never PSUM recycling.
            for b in range(N_BLK):
                r = b % XR
                for img in range(IMGS_PER_CORE):
                    v.wait_ge(s_xb[img][r], 16 * len(DMA_PIECES) * (b // XR + 1))
                for img in range(IMGS_PER_CORE):
                    # dy=2 im2col replica: xb[64:96, s] = xb[32:64, s+1]
                    v.tensor_copy(
                        out=xb[img][r][64:96, 0:BLK, :],
                        in_=xb[img][r][32:64, 1 : BLK + 1, :],
                    ).then_inc(s_cp, 1)
                if b >= 1:
                    _evac_dve(v, b - 1)
            _evac_dve(v, N_BLK - 1)

        @block.tensor
        def _(t):
            t.wait_ge(s_wt, 16)
            for b in range(N_BLK):
                r = b % XR
                for img in range(IMGS_PER_CORE):
                    t.wait_ge(s_xb[img][r], 16 * len(DMA_PIECES) * (b // XR + 1))
                t.wait_ge(s_cp, IMGS_PER_CORE * (b + 1))
                for p in range(PPB):
                    gp = b * PPB + p
                    if gp >= NPS:
                        # psum bank reuse: evac of row-pair gp-NPS done
                        tb, tp = divmod(gp - NPS, PPB)
                        if tp < EVS_PPB:
                            t.wait_ge(s_evs, EVS_PPB * tb + tp + 1)
                        else:
                            t.wait_ge(s_evv, EVV_PPB * tb + tp - EVS_PPB + 1)
                    bank = ps[gp % NPS]
                    b0 = 2 * p
                    last = None
                    for dx in range(3):
                        for img in range(IMGS_PER_CORE):
                            last = nc.tensor.matmul(
                                bank[img * OC : (img + 1) * OC, :, :],
                                wt_sb[:, dx, :],
                                xb[img][r][:, b0 : b0 + 2, dx : dx + W],
                                start=dx == 0,
                                stop=dx == 2,
                                skip_group_check=True,
                            )
                    last.then_inc(s_mm, 1)

        @block.scalar
        def _(sc):
            sc.wait_ge(s_bias, 16)
            for b in range(N_BLK):
                ob = outb[b % OR]
                for p in range(0, EVS_PPB):
                    gp = b * PPB + p
                    if b >= OR and p == 0:
                        sc.wait_ge(s_yo[b % OR][0], 16 * ((b - OR) // OR + 1))
                    if b >= OR and p == 7:
                        sc.wait_ge(s_yo[b % OR][1], 16 * ((b - OR) // OR + 1))
                    sc.wait_ge(s_mm, gp + 1)
                    sc.activation(
                        ob[:, 2 * p : 2 * p + 2, :],
                        ps[gp % NPS][:, :, :],
                        mybir.ActivationFunctionType.Identity,
                        bias=bias_sb[:, :],
                    ).then_inc(s_evs, 1)

    return nc


def _get_nc() -> bass.Bass:
    if "nc" not in _nc_cache:
        _nc_cache["nc"] = _build_nc()
    return _nc_cache["nc"]


def kernel(x: np.ndarray, weight: np.ndarray, bias: np.ndarray) -> np.ndarray:
    global LAST_EXEC_NS, LAST_RESULTS
    import ml_dtypes

    n = x.shape[0]
    assert n == N_CORES * IMGS_PER_CORE

    in_np = ml_dtypes.bfloat16
    xp = np.zeros((n, IC, HP, WP), dtype=in_np)
    xp[:, :, 1 : H + 1, 1 : W + 1] = x
    # WT[dy*32+ic, dx, oc] = weight[oc, ic, dy, dx]
    wt = np.ascontiguousarray(weight.transpose(2, 1, 3, 0).reshape(96, 3, OC)).astype(
        in_np
    )
    b2 = np.ascontiguousarray(np.tile(bias.reshape(OC, 1), (2, 1))).astype(np.float32)

    nc = _get_nc()
    in_maps = [
        {
            "x": np.ascontiguousarray(xp[i * IMGS_PER_CORE : (i + 1) * IMGS_PER_CORE]),
            "wt": wt,
            "bias": b2,
        }
        for i in range(N_CORES)
    ]
    if TRACE:
        _install_ntff_shim()
    res = run_bass_kernel_spmd(nc, in_maps, core_ids=list(range(N_CORES)), trace=TRACE)
    LAST_EXEC_NS = res.exec_time_ns
    LAST_RESULTS = res
    y = np.concatenate([r["y"] for r in res.results], axis=0)
    return y.astype(np.float32)


# revision 15
# speedup vs baseline: 1.4937x; 1.0236x over previous
"""Conv2D 3x3 (stride 1, pad 1) NCHW/OIHW, data-parallel over 8 NeuronCores.

Full inputs: x (16,32,224,224) f32, weight (64,32,3,3) f32, bias (64,) f32.
Full output: (16,64,224,224) f32.

Raw-Bass SPMD kernel, per core (2 images).  The 224 output rows are cut
into blocks of [14, 28 x7, 14] rows: the small edge blocks halve the
pipeline-fill (first matmul needs only a 16-row load + copies) and the
drain (last evac+store chain).  Per block:

  - The dy=0/dy=1 groups of the im2col buffer XB[img][p = dy*32 + ic, s, c]
    = xpad[img, ic, i0+s+dy, c] are DMA'd from HBM; DVE builds the dy=2
    group as a row-shifted copy of dy=1 (which loads one extra row).  The
    DRAM-side AP has the 32-wide ic dim outermost, so the descriptor
    generator sprays each load round-robin across all 16 SDMA engines, in
    <=15-row pieces so every packet stays <=6.8KB (the engines move ~25GB/s
    on ~6KB packets but drop to ~14GB/s on >8KB ones; HBM-read packets cost
    ~500ns when reads run alone and ~240ns once store packets interleave).
  - Each output row-pair = 3 PSUM-accumulated matmuls (K=96, M=64, N=448),
    dx realized as a free-dim offset; the two images ride different PE
    column groups (PSUM partitions 0-63 / 64-127) and stream concurrently.
    Floor: 2 streams x 1 elem/cycle @2.4GHz = 557ns per row-pair.
  - PSUM evac (+bias, f32->bf16): ScalarE (activation) takes 10 row-pairs
    per 14-pair block, DVE (tensor_scalar_add) the other 4.  DVE's evacs of
    block b run after the copies of block b+1, so only pairs whose PSUM
    bank is re-needed in a LATER block may go to DVE (pairs 6..13 of a
    14-pair block); within that range the engines alternate so no run of
    consecutive banks is freed at ScalarE's 634ns/evac (above PE's 557ns).
  - Engine/queue assignment decouples the pipelines: GpSimd issues input
    DMAs on its own hardware queue (never behind stores in a FIFO, never
    behind evac waits); SP emits stores; prologue loads for block 0 ride
    SP's and ScalarE's queues so they are processed before the ramp-up
    loads GpSimd is streaming.
  - Output rides HBM as bf16 (halves store traffic; measured end-to-end rel
    err ~3.6e-3 vs the 2e-2 gate) and is widened to f32 on the host.
  - All cross-engine sync is explicit semaphores; DMA-sem waiters always
    wait the full +16-per-DMA count (sound under out-of-order per-SDMA
    completion).  The local walrus rejects multi-wait instructions, so
    every wait is its own instruction.
"""

import sys

sys.path.insert(0, "/opt/trn_rl_repo")

from contextlib import ExitStack

import numpy as np

import concourse.bass as bass
from concourse import mybir
from concourse.bass_utils import run_bass_kernel_spmd

N_CORES = 8
IMGS_PER_CORE = 2
IC, OC, H, W = 32, 64, 224, 224
HP, WP = 226, 226  # padded
SLOTS = 29  # xb row-slots (max block: 28 rows + 1 extra dy=1 row)
XR = 5  # xb ring depth
OR = 3  # out ring depth
NPS = 8  # psum banks in rotation

# blocks of output rows: small edges for fast fill/drain
BLOCK_ROWS = [14, 28, 28, 28, 28, 28, 28, 28, 14]
assert sum(BLOCK_ROWS) == H
N_BLK = len(BLOCK_ROWS)
BLK_I0 = [sum(BLOCK_ROWS[:b]) for b in range(N_BLK)]
BLK_NP = [r // 2 for r in BLOCK_ROWS]  # row-pairs per block
CUM_NP = [sum(BLK_NP[:b]) for b in range(N_BLK + 1)]  # pair offset per block


def _dve_pairs(b):
    # DVE may only own pairs whose PSUM bank is re-needed in a later block
    # (DVE's evacs of block b trail the copies of block b+1); block 0 stays
    # all-ScalarE so its evacs never wait on the ramp.  Alternate with
    # ScalarE elsewhere so no run of consecutive banks is ScalarE-paced.
    if b == 0:
        return ()
    if BLK_NP[b] == 14:
        return (7, 9, 11, 13)
    return (1, 3, 5)


DVE_PAIRS = [_dve_pairs(b) for b in range(N_BLK)]
SC_PAIRS = [
    [p for p in range(BLK_NP[b]) if p not in DVE_PAIRS[b]] for b in range(N_BLK)
]
# _owner[gp] = (is_dve, cumulative count on the owning engine)
_owner = []
_cs = _cv = 0
for _b in range(N_BLK):
    for _p in range(BLK_NP[_b]):
        if _p in DVE_PAIRS[_b]:
            _cv += 1
            _owner.append((True, _cv))
        else:
            _cs += 1
            _owner.append((False, _cs))

# input DMA pieces per block: (dy, slot0, n_rows), every piece <=15 rows
DMA_PIECES = []
for _b in range(N_BLK):
    _rows = BLOCK_ROWS[_b]
    _pcs = []
    for _dy, _nr in ((0, _rows), (1, _rows + 1)):
        if _nr <= 15:
            _pcs.append((_dy, 0, _nr))
        else:
            _h0 = (_nr + 1) // 2
            _pcs.append((_dy, 0, _h0))
            _pcs.append((_dy, _h0, _nr - _h0))
    DMA_PIECES.append(_pcs)
# s_xb cumulative count (per img) after block b's loads: +16 per DMA, the
# ring slot b%XR accumulates across its rounds
XB_CNT = [
    16 * sum(len(DMA_PIECES[_bb]) for _bb in range(_b + 1) if _bb % XR == _b % XR)
    for _b in range(N_BLK)
]
# s_yo bookkeeping: block b half h waits for 16 * (# prior stores on
# (slot b%OR, h)); 7-pair blocks store only half 0
_yo_seen = {}
YO_PRIOR = []
for _b in range(N_BLK):
    _halves = (0,) if BLK_NP[_b] == 7 else (0, 1)
    _pri = {}
    for _h in _halves:
        _k = (_b % OR, _h)
        _pri[_h] = _yo_seen.get(_k, 0)
        _yo_seen[_k] = _pri[_h] + 1
    YO_PRIOR.append(_pri)
YO_TOTAL = dict(_yo_seen)

DT_MODE = "bf16"  # kept for test.py compat; only bf16 is supported

TRACE = False  # test.py can flip this to get LAST_EXEC_NS
LAST_EXEC_NS = None
LAST_RESULTS = None

_nc_cache = {}


def _install_ntff_shim():
    """The agent image's antenv lacks axon_hooks; recreate the NTFF profile
    hook via ctypes against libaxon_pjrt.so (same ABI trn_boot.py uses)."""
    try:
        import antenv.axon_hooks  # noqa: F401

        return
    except ImportError:
        pass
    import contextlib
    import ctypes
    import types

    so_path = "/opt/axon/libaxon_pjrt.so"
    lib = ctypes.CDLL(so_path)
    if not hasattr(lib, "axon_start_nrt_profile"):
        return
    lib.axon_start_nrt_profile.argtypes = [
        ctypes.POINTER(ctypes.c_int64),
        ctypes.c_size_t,
    ]
    lib.axon_start_nrt_profile.restype = ctypes.c_int64
    lib.axon_stop_nrt_profile.argtypes = [ctypes.c_char_p]
    lib.axon_stop_nrt_profile.restype = ctypes.c_int64

    @contextlib.contextmanager
    def _hook(output_dir, device_ids):
        import jax

        jax.devices()
        if device_ids:
            ids = (ctypes.c_int64 * len(device_ids))(*device_ids)
            rc = lib.axon_start_nrt_profile(ids, len(device_ids))
        else:
            rc = lib.axon_start_nrt_profile(None, 0)
        if rc != 0:
            raise RuntimeError(f"axon_start_nrt_profile rc={rc}")
        try:
            yield
        finally:
            n = lib.axon_stop_nrt_profile(str(output_dir).encode())
            print(f"ntff profile: {n} file(s) written to {output_dir}")

    mod = types.ModuleType("antenv.axon_hooks")
    mod.get_axon_ntff_profile_hook = lambda: _hook
    mod.set_axon_ntff_profile_hook = lambda h: None
    import antenv

    sys.modules["antenv.axon_hooks"] = mod
    antenv.axon_hooks = mod


def _build_nc() -> bass.Bass:
    f32 = mybir.dt.float32
    bf16 = mybir.dt.bfloat16

    nc = bass.Bass()
    x = nc.dram_tensor("x", [IMGS_PER_CORE, IC, HP, WP], bf16, kind="ExternalInput")
    wt = nc.dram_tensor("wt", [96, 3, OC], bf16, kind="ExternalInput")
    bias = nc.dram_tensor("bias", [128, 1], f32, kind="ExternalInput")
    y = nc.dram_tensor("y", [IMGS_PER_CORE, OC, H, W], bf16, kind="ExternalOutput")

    ctx = ExitStack()
    wt_sb = ctx.enter_context(nc.sbuf_tensor("wt_sb", [96, 3, OC], bf16))
    bias_sb = ctx.enter_context(nc.sbuf_tensor("bias_sb", [128, 1], f32))
    xb = [
        [
            ctx.enter_context(nc.sbuf_tensor(f"xb_{i}_{r}", [96, SLOTS, WP], bf16))
            for r in range(XR)
        ]
        for i in range(IMGS_PER_CORE)
    ]
    outb = [
        ctx.enter_context(nc.sbuf_tensor(f"outb_{s}", [128, 28, W], bf16))
        for s in range(OR)
    ]
    ps = [
        ctx.enter_context(nc.psum_tensor(f"ps_{k}", [128, 2, W], f32))
        for k in range(NPS)
    ]

    s_wt = ctx.enter_context(nc.semaphore("s_wt"))
    s_bias = ctx.enter_context(nc.semaphore("s_bias"))
    s_xb = [
        [ctx.enter_context(nc.semaphore(f"s_xb_{i}_{r}")) for r in range(XR)]
        for i in range(IMGS_PER_CORE)
    ]
    s_yo = [
        [ctx.enter_context(nc.semaphore(f"s_yo_{s}_{h}")) for h in range(2)]
        for s in range(OR)
    ]
    s_cp = ctx.enter_context(nc.semaphore("s_cp"))
    s_mm = ctx.enter_context(nc.semaphore("s_mm"))
    s_evs = ctx.enter_context(nc.semaphore("s_evs"))
    s_evv = ctx.enter_context(nc.semaphore("s_evv"))

    st_img = IC * HP * WP
    st_ic = HP * WP

    def piece_src(img, i0, dy, s0, n_rows):
        # partition = ic (32-wide, outermost -> 16-engine DMA spray);
        # free (s, c); each partition reads n_rows*WP contiguous elements.
        return bass.AP(
            tensor=x[0, 0, 0:1, 0:1].tensor,
            offset=img * st_img + (i0 + dy + s0) * WP,
            ap=[[st_ic, IC], [WP, n_rows], [1, WP]],
        )

    with ctx, nc.Block() as block:

        def _issue_inputs(eng, b, img):
            r = b % XR
            i0 = BLK_I0[b]
            if b >= XR and img == 0:
                # xb slot reuse: PE matmuls of block b-XR done (the DVE dy=2
                # copy of b-XR precedes PE b-XR via s_cp, so this also
                # covers the copy's read of the dy=1 rows).
                eng.wait_ge(s_mm, CUM_NP[b - XR + 1])
            for dy, s0, n_rows in DMA_PIECES[b]:
                eng.dma_start(
                    out=xb[img][r][dy * 32 : (dy + 1) * 32, s0 : s0 + n_rows, :],
                    in_=piece_src(img, i0, dy, s0, n_rows),
                ).then_inc(s_xb[img][r], 16)

        @block.gpsimd
        def _(g):
            # all steady-state input DMA issue: its own hardware queue, so
            # input transfers never sit behind stores on a shared FIFO and
            # issue never sits behind store/evac waits.
            for b in range(N_BLK):
                for img in range(IMGS_PER_CORE):
                    if b == 0:
                        continue  # block 0 rides SP's + ScalarE's queues
                    if b == 1 and img == 1:
                        continue  # rides ScalarE's queue
                    _issue_inputs(g, b, img)

        @block.sync
        def _(sync):
            def emit_out(b):
                i0 = BLK_I0[b]
                ob = outb[b % OR]
                # h=0: rows 0..13 = pairs 0..6 (mixed-engine in 7-pair blocks)
                rng = _owner[CUM_NP[b] : CUM_NP[b] + min(7, BLK_NP[b])]
                sync.wait_ge(s_evs, max(c for d, c in rng if not d))
                dv0 = max((c for d, c in rng if d), default=0)
                if dv0:
                    sync.wait_ge(s_evv, dv0)
                sync.dma_start(
                    out=y[:, :, i0 : i0 + 14, :],
                    in_=ob[:, 0:14, :],
                ).then_inc(s_yo[b % OR][0], 16)
                if BLK_NP[b] == 7:
                    return
                # h=1: rows 14..27 = pairs 7..13
                end = CUM_NP[b + 1]
                sc_cnt = max(c for d, c in _owner[:end] if not d)
                dv_cnt = max((c for d, c in _owner[:end] if d), default=0)
                sync.wait_ge(s_evs, sc_cnt)
                if dv_cnt:
                    sync.wait_ge(s_evv, dv_cnt)
                sync.dma_start(
                    out=y[:, :, i0 + 14 : i0 + 28, :],
                    in_=ob[:, 14:28, :],
                ).then_inc(s_yo[b % OR][1], 16)

            sync.dma_start(out=wt_sb[:, :, :], in_=wt[:, :, :]).then_inc(s_wt, 16)
            sync.dma_start(out=bias_sb[:, :], in_=bias[:, :]).then_inc(s_bias, 16)
            # prologue ramp: block 0's img0 load rides SP's queue so it is
            # processed ahead of the ramp-up loads GpSimd streams on its
            # queue (engines round-robin across queues, so a dedicated
            # queue = high priority while stores haven't started).
            _issue_inputs(sync, 0, 0)
            for b in range(N_BLK):
                if b >= 1:
                    emit_out(b - 1)
            emit_out(N_BLK - 1)
            for (s, h), n in sorted(YO_TOTAL.items()):
                sync.wait_ge(s_yo[s][h], 16 * n)

        def _evac_dve(v, b):
            ob = outb[b % OR]
            for p in DVE_PAIRS[b]:
                gp = CUM_NP[b] + p
                # outb slot reuse (DVE writes h1 rows of 14-pair blocks,
                # h0 rows of 7-pair blocks)
                h = 1 if BLK_NP[b] == 14 else 0
                if p == DVE_PAIRS[b][0] and YO_PRIOR[b].get(h, 0) > 0:
                    v.wait_ge(s_yo[b % OR][h], 16 * YO_PRIOR[b][h])
                v.wait_ge(s_mm, gp + 1)
                v.tensor_scalar_add(
                    ob[:, 2 * p : 2 * p + 2, :],
                    ps[gp % NPS][:, :, :],
                    bias_sb[:, :],
                ).then_inc(s_evv, 1)

        @block.vector
        def _(v):
            v.wait_ge(s_bias, 16)
            # copies for block b run first (inputs arrive well ahead), then
            # evacs of block b-1 drain as PE produces them -- so a late
            # input can only delay PE block b's start, never PSUM recycling.
            for b in range(N_BLK):
                r = b % XR
                rows = BLOCK_ROWS[b]
                for img in range(IMGS_PER_CORE):
                    # per-img wait so img0's copy overlaps img1's transfer
                    v.wait_ge(s_xb[img][r], XB_CNT[b])
                    # dy=2 im2col replica: xb[64:96, s] = xb[32:64, s+1]
                    v.tensor_copy(
                        out=xb[img][r][64:96, 0:rows, :],
                        in_=xb[img][r][32:64, 1 : rows + 1, :],
                    ).then_inc(s_cp, 1)
                if b >= 1:
                    _evac_dve(v, b - 1)
            _evac_dve(v, N_BLK - 1)

        @block.tensor
        def _(t):
            t.wait_ge(s_wt, 16)
            for b in range(N_BLK):
                r = b % XR
                for img in range(IMGS_PER_CORE):
                    t.wait_ge(s_xb[img][r], XB_CNT[b])
                t.wait_ge(s_cp, IMGS_PER_CORE * (b + 1))
                for p in range(BLK_NP[b]):
                    gp = CUM_NP[b] + p
                    if gp >= NPS:
                        # psum bank reuse: evac of row-pair gp-NPS done
                        is_dve, cnt = _owner[gp - NPS]
                        t.wait_ge(s_evv if is_dve else s_evs, cnt)
                    bank = ps[gp % NPS]
                    b0 = 2 * p
                    last = None
                    for dx in range(3):
                        for img in range(IMGS_PER_CORE):
                            last = nc.tensor.matmul(
                                bank[img * OC : (img + 1) * OC, :, :],
                                wt_sb[:, dx, :],
                                xb[img][r][:, b0 : b0 + 2, dx : dx + W],
                                start=dx == 0,
                                stop=dx == 2,
                                skip_group_check=True,
                            )
                    last.then_inc(s_mm, 1)

        @block.scalar
        def _(sc):
            # prologue: img1 loads for blocks 0..1 ride the scalar engine's
            # own hardware DMA queue -- a third queue in flight while the
            # pipeline fills, and block 0's packets are not diluted by the
            # ramp-up loads on GpSimd's queue.
            for b in range(2):
                _issue_inputs(sc, b, 1)
            sc.wait_ge(s_bias, 16)
            for b in range(N_BLK):
                ob = outb[b % OR]
                h1_first = next((q for q in SC_PAIRS[b] if q >= 7), None)
                for p in SC_PAIRS[b]:
                    gp = CUM_NP[b] + p
                    if p == SC_PAIRS[b][0] and YO_PRIOR[b].get(0, 0) > 0:
                        sc.wait_ge(s_yo[b % OR][0], 16 * YO_PRIOR[b][0])
                    if p == h1_first and YO_PRIOR[b].get(1, 0) > 0:
                        sc.wait_ge(s_yo[b % OR][1], 16 * YO_PRIOR[b][1])
                    sc.wait_ge(s_mm, gp + 1)
                    sc.activation(
                        ob[:, 2 * p : 2 * p + 2, :],
                        ps[gp % NPS][:, :, :],
                        mybir.ActivationFunctionType.Identity,
                        bias=bias_sb[:, :],
                    ).then_inc(s_evs, 1)

    return nc


def _get_nc() -> bass.Bass:
    if "nc" not in _nc_cache:
        _nc_cache["nc"] = _build_nc()
    return _nc_cache["nc"]


def kernel(x: np.ndarray, weight: np.ndarray, bias: np.ndarray) -> np.ndarray:
    global LAST_EXEC_NS, LAST_RESULTS
    import ml_dtypes

    n = x.shape[0]
    assert n == N_CORES * IMGS_PER_CORE

    in_np = ml_dtypes.bfloat16
    xp = np.zeros((n, IC, HP, WP), dtype=in_np)
    xp[:, :, 1 : H + 1, 1 : W + 1] = x
    # WT[dy*32+ic, dx, oc] = weight[oc, ic, dy, dx]
    wt = np.ascontiguousarray(weight.transpose(2, 1, 3, 0).reshape(96, 3, OC)).astype(
        in_np
    )
    b2 = np.ascontiguousarray(np.tile(bias.reshape(OC, 1), (2, 1))).astype(np.float32)

    nc = _get_nc()
    in_maps = [
        {
            "x": np.ascontiguousarray(xp[i * IMGS_PER_CORE : (i + 1) * IMGS_PER_CORE]),
            "wt": wt,
            "bias": b2,
        }
        for i in range(N_CORES)
    ]
    if TRACE:
        _install_ntff_shim()
    res = run_bass_kernel_spmd(nc, in_maps, core_ids=list(range(N_CORES)), trace=TRACE)
    LAST_EXEC_NS = res.exec_time_ns
    LAST_RESULTS = res
    y = np.concatenate([r["y"] for r in res.results], axis=0)
    return y.astype(np.float32)
